# revision 49
# baseline (speedup 1.0000x reference)
"""AdaptiveGCN (2-layer GCNConv + BN eval + adaptive relu/gelu blend) on 8 TRN2 cores.

v4 strategy (chunk-pipelined gather):
  The hard floor on TRN2 is the pass-2 device gather: every in-edge needs one
  SWDGE descriptor pair (~8.4ns/edge of GpSimd ucode time), so the per-core
  gather of E/8 = 75k table rows costs ~650us no matter how it's expressed
  (dma_gather / indirect_dma_start / ap_gather all measured at or above this
  rate). v4 therefore hides *everything else* under the gather stream:

  - Nodes are dst-sharded; each core's 49 dst blocks are grouped into 4
    fixed chunks (quarters of the node range, permutation-independent).
  - Pass 1 (conv1+BN+blend) runs chunk by chunk; each chunk's table slice
    (h2 = y1 @ W1*s1, transposed per block on PE) is stored and AllGathered
    immediately, so pass-2 gathers of chunk 0 start ~55us into the kernel
    while pass 1 of chunks 1-3 still runs on PE/DVE/ACT.
  - Pass-2 accumulation is chunk-major: per (chunk, block) a short psum
    accumulation over edge slot tiles (one-hot mew matmuls), then a DVE add
    into a persistent f32 out_acc region (adds commute - no ordering needed).
  - Self-loops never enter the gather: at table-build time the block's own
    (transposed) table psum is scaled by dinv^2 and added into out_acc.
  - deg/dinv are host-precomputed from (edge_index, edge_weight) and folded
    into the streamed one-hot weights (mew1/mew2), killing the v3 dsw/dgl
    streams and the per-chunk DVE broadcast scaling.
  - BN folds: scale into W0/W1 (bf16), bias via scalar-engine activation
    (bias/scale per partition; psum orientation is [feat, dst] in pass 1).
  - AllGather is split into 4 sub-collectives; each trigger is emitted
    between gather calls of the previous chunk so GpSimd never stalls.
"""

import dataclasses
import ml_dtypes
import numpy as np
from contextlib import ExitStack

from concourse import bass, bacc, mybir, tile, library_config
from concourse.bass_utils import run_bass_kernel_spmd

F32 = mybir.dt.float32
BF16 = mybir.dt.bfloat16
I16 = mybir.dt.int16
I32 = mybir.dt.int32
AF = mybir.ActivationFunctionType
OP = mybir.AluOpType
AX = mybir.AxisListType


@dataclasses.dataclass
class Cfg:
    N: int = 50000
    E: int = 600000
    D: int = 128
    P: int = 8            # cores
    BLK: int = 128        # dst nodes per block
    GM: int = 32          # stream slots per chunk-load
    GS: int = 8           # slots per dma_gather (1024-idx ucode cap)
    bn_eps: float = 1e-5
    chunk_blocks: tuple = (9, 12, 13, 15)    # dst-block counts per pipeline chunk
    trig_fracs: tuple = (0.25, 0.25, 0.25)  # AG q+1 trigger position in phase q


# ---------------------------------------------------------------- host prep

def host_prep(x, edge_index, edge_weight, cfg: Cfg):
    N, E, P, BLK = cfg.N, cfg.E, cfg.P, cfg.BLK
    NL = N // P
    NB = (NL + BLK - 1) // BLK
    CB = list(cfg.chunk_blocks)
    NC = len(CB)
    assert sum(CB) == NB
    # block sizes (last block is short)
    bsz = np.full(NB, BLK, np.int64)
    bsz[NB - 1] = NL - BLK * (NB - 1)
    cb_lo = np.concatenate([[0], np.cumsum(CB)]).astype(np.int64)   # chunk -> first block
    # rows per chunk (local), chunk row offsets (local)
    rows_q = np.array([bsz[cb_lo[q]:cb_lo[q + 1]].sum() for q in range(NC)], np.int64)
    roff_q = np.concatenate([[0], np.cumsum(rows_q)]).astype(np.int64)
    # global table layout: [AG0: P*rows_0 | AG1: P*rows_1 | ...]
    goff_q = np.concatenate([[0], np.cumsum(P * rows_q)]).astype(np.int64)
    assert all(P * rows_q <= 32767)

    srcE = edge_index[0].astype(np.int64)
    dstE = edge_index[1].astype(np.int64)
    ewE = edge_weight.astype(np.float64)

    # host deg/dinv (pure input math; self-loop weight 1)
    deg = np.bincount(dstE, weights=ewE, minlength=N) + 1.0
    dinv = 1.0 / np.sqrt(deg)

    # chunk of a node: by its *natural* local position (fixed, perm-independent)
    loc_all = np.arange(N) % NL
    chunk_of = np.searchsorted(roff_q[1:], loc_all, side="right")  # [N] in [0, NC)

    # ---- per-core greedy: assign local nodes to blocks inside their chunk,
    # balancing the NC per-chunk in-edge counts + total (incl self-loop).
    core_d = dstE // NL
    kcnt = np.zeros((N, NC), np.float64)  # in-edges of node d from src-chunk k
    np.add.at(kcnt, (dstE, chunk_of[srcE]), 1.0)

    perms = []
    for c in range(P):
        pnew = np.zeros(NL, np.int64)
        for q in range(NC):
            nlo, nhi = roff_q[q], roff_q[q + 1]
            nodes = np.arange(nlo, nhi)              # local ids in this chunk
            kc = kcnt[c * NL + nodes]                # [nq, NC]
            tot = kc.sum(axis=1) + 1.0
            nb = CB[q]
            caps = bsz[cb_lo[q]:cb_lo[q + 1]].astype(np.int64)
            # per-(block,chunk) quotas: stagger tile counts so most cells sit
            # just under a 128 boundary (26-edge margin for cross-core jitter)
            dem = kc.sum(axis=0)                     # per-chunk demand in this quarter
            quota = np.zeros((nb, NC))
            for k in range(NC):
                m = dem[k] / nb
                t_hi = int(np.ceil((m + 26.0) / 128.0))
                cap_hi = t_hi * 128.0 - 26.0
                # how many blocks can drop one tile while still fitting
                nshrink = 0
                if t_hi > 1:
                    cap_lo = (t_hi - 1) * 128.0 - 26.0
                    nshrink = int(min(nb - 1, max(0.0, (nb * cap_hi - dem[k] * 1.03) // (cap_hi - cap_lo))))
                col = np.full(nb, cap_hi)
                col[:nshrink] = (t_hi - 1) * 128.0 - 26.0
                quota[:, k] = np.roll(col, k)        # stagger across chunks
            tq = (tot.sum() + nb - 1) / nb + 20.0
            order_n = np.argsort(-tot, kind="stable")
            fill = np.zeros(nb, np.int64)
            L = np.zeros((nb, NC)); Tt = np.zeros(nb)
            for n in order_n:
                cost = np.max((L + kc[n]) / quota, axis=1)
                cost = np.maximum(cost, (Tt + tot[n]) / (13.0 * 128.0))
                cost = np.maximum(cost, (Tt + tot[n]) / tq * 0.01)
                cost[fill >= caps] = 1e9
                bi = int(np.argmin(cost))
                pnew[nodes[n]] = (cb_lo[q] + bi) * BLK + fill[bi]
                fill[bi] += 1
                L[bi] += kc[n]; Tt[bi] += tot[n]
        perms.append(pnew)

    # global table row of node n: chunk q region + core stripe + chunk offset
    tpos_g = np.zeros(N, np.int64)
    for c in range(P):
        pn = perms[c]                    # local node -> new local pos (block*128+off, block-major)
        q = chunk_of[c * NL + np.arange(NL)]
        posq = pn - cb_lo[q] * BLK       # position within own chunk (block-major)
        tpos_g[c * NL:(c + 1) * NL] = goff_q[q] + c * rows_q[q] + posq
    x_bf = np.asarray(x, np.float32).astype(ml_dtypes.bfloat16)
    dinv32 = dinv.astype(np.float32)

    # ---------------- pass-1 schedule: edges + self-loops, sorted (block, dst)
    wl1 = (dinv[srcE] * ewE * dinv[dstE]).astype(np.float32)
    src1 = np.concatenate([srcE, np.arange(N, dtype=np.int64)])
    dst1 = np.concatenate([dstE, np.arange(N, dtype=np.int64)])
    # self-loop pass-1 weight would be dinv^2 but self term is added at table
    # time in pass 2 only; pass 1 needs it in the aggregation: conv1 includes
    # self-loops, so keep them in the pass-1 stream.
    w1 = np.concatenate([wl1, (dinv * dinv).astype(np.float32)])
    core1 = dst1 // NL
    per1, counts1 = [], np.zeros((P, NB), np.int64)
    for c in range(P):
        m = core1 == c
        s_, d_, w_ = src1[m], perms[c][dst1[m] - c * NL], w1[m]
        b_ = d_ // BLK
        o = np.lexsort((d_, b_))
        per1.append((s_[o], d_[o], w_[o], b_[o]))
        counts1[c] = np.bincount(b_, minlength=NB)
    tiles1 = np.ceil(counts1.max(axis=0) / 128).astype(np.int64)
    T1 = int(tiles1.sum())
    sbase1 = np.concatenate([[0], np.cumsum(tiles1)]).astype(np.int64)
    pad1 = (T1 * 128 * P - counts1.sum()) / counts1.sum()

    # ---------------- pass-2 schedule: real edges, dense per-block tiling.
    # Edges sorted (block, src-chunk, dst) and packed densely into each
    # block's lanes; a 128-lane tile may straddle a chunk boundary. Each tile
    # gets a COMMON phase = max src-chunk present in it across cores, and is
    # gathered from a two-chunk table window [goff[p-1], goff[p+1]) so the
    # straddle costs no padding. Slot order is (phase, block, tile).
    w2 = (dinv[srcE] * ewE * dinv[dstE]).astype(np.float32)
    k2 = chunk_of[srcE]
    per2 = []
    counts2 = np.zeros((P, NB), np.int64)            # per-block totals
    cum2 = np.zeros((P, NB, NC + 1), np.int64)       # cumulative per chunk
    for c in range(P):
        m = core_d == c
        s_, d_, w_, k_ = tpos_g[srcE[m]], perms[c][dstE[m] - c * NL], w2[m], k2[m]
        b_ = d_ // BLK
        o = np.lexsort((d_, k_, b_))
        s_, d_, w_, k_, b_ = s_[o], d_[o], w_[o], k_[o], b_[o]
        per2.append((s_, d_, w_, k_, b_))
        counts2[c] = np.bincount(b_, minlength=NB)
        for q in range(NC):
            cum2[c, :, q + 1] = cum2[c, :, q] + np.bincount(b_[k_ == q], minlength=NB)
    tiles2 = np.ceil(counts2.max(axis=0) / 128).astype(np.int64)  # [NB]
    T2 = int(tiles2.sum())
    pad2 = (T2 * 128 * P - counts2.sum()) / counts2.sum()

    # phase of each tile: smallest p s.t. cross-core max cum through chunk p
    # covers the tile start... practically: max over cores of the chunk of the
    # last lane the core has in this tile (cores without content contribute 0)
    phase_b = []                                      # [NB] -> array [tiles_b]
    for b in range(NB):
        tb = int(tiles2[b])
        ph = np.zeros(tb, np.int64)
        for t in range(tb):
            hi_lane = (t + 1) * 128 - 1
            p = 0
            pmin = NC
            for c in range(P):
                tot = counts2[c, b]
                if tot <= t * 128:
                    continue
                last = min(hi_lane, tot - 1)
                pc = int(np.searchsorted(cum2[c, b, 1:], last, side="right"))
                p = max(p, pc)
                pc0 = int(np.searchsorted(cum2[c, b, 1:], t * 128, side="right"))
                pmin = min(pmin, pc0)
            ph[t] = p
            if pmin < NC and pmin < p - 1:
                raise RuntimeError(f"tile spans >2 chunks: b={b} t={t} {pmin} {p}")
        phase_b.append(ph)
    # slot order (phase, block, tile): per (phase, block) runs
    rbase = np.full((NC, NB), -1, np.int64)
    rlen = np.zeros((NC, NB), np.int64)
    slot_of = [np.zeros(int(tiles2[b]), np.int64) for b in range(NB)]
    acc = 0
    chunk_slot_lo = np.zeros(NC + 1, np.int64)
    for q in range(NC):
        chunk_slot_lo[q] = acc
        for b in range(NB):
            ts = np.nonzero(phase_b[b] == q)[0]
            if len(ts) == 0:
                continue
            rbase[q, b] = acc
            rlen[q, b] = len(ts)
            for t in ts:
                slot_of[b][t] = acc
                acc += 1
    chunk_slot_lo[NC] = acc
    assert acc == T2
    tile_base = np.concatenate([[0], np.cumsum(tiles2)]).astype(np.int64)
    slot_flat = np.concatenate(slot_of) if T2 else np.zeros(0, np.int64)
    phase_flat = np.concatenate(phase_b) if T2 else np.zeros(0, np.int64)
    # gather view per phase: [view_lo[q], goff_q[q+1])
    view_lo = goff_q[np.maximum(np.arange(NC) - 1, 0)]

    in_maps = []
    for c in range(P):
        # pass 1 arrays
        s_, d_, w_, b_ = per1[c]
        bs = np.concatenate([[0], np.cumsum(counts1[c])]).astype(np.int64)
        p_ = np.arange(len(b_)) - bs[b_]
        lane, sl = p_ % 128, sbase1[b_] + p_ // 128
        xe = np.zeros((128, T1, 128), ml_dtypes.bfloat16)
        xe[lane, sl, :] = x_bf[s_]
        mew1 = np.zeros((128, T1, 128), ml_dtypes.bfloat16)
        mew1[lane, sl, d_ % BLK] = w_.astype(ml_dtypes.bfloat16)

        # pass 2 arrays (dense per-block lanes, phase-major slots)
        s_, d_, w_, k_, b_ = per2[c]
        bs2 = np.concatenate([[0], np.cumsum(counts2[c])]).astype(np.int64)
        p_ = np.arange(len(b_)) - bs2[b_]
        lane = p_ % 128
        t_ = p_ // 128
        sl = slot_flat[tile_base[b_] + t_]
        ph = phase_flat[tile_base[b_] + t_]
        mew2 = np.zeros((128, T2, 128), ml_dtypes.bfloat16)
        mew2[lane, sl, d_ % BLK] = w_.astype(ml_dtypes.bfloat16)
        idx = np.zeros((16, 8 * T2), np.int16)
        idxval = s_ - view_lo[ph]
        assert idxval.min() >= 0 and idxval.max() <= 32767
        idx[lane % 16, sl * 8 + lane // 16] = idxval.astype(np.int16)

        # self-loop weights dinv^2 in (block, pos) layout
        wself = np.zeros((128, NB), np.float32)
        npos = np.arange(NL)
        newp = perms[c][npos]
        dv = dinv32[c * NL + npos]
        wself[newp % BLK, newp // BLK] = dv * dv

        in_maps.append({
            "xe": xe.reshape(128, T1 * 128),
            "mew1": mew1.reshape(128, T1 * 128),
            "mew2": mew2.reshape(128, T2 * 128),
            "idx": np.tile(idx, (8, 1)),
            "wself": wself,
        })

    meta = dict(NL=NL, NB=NB, NC=NC, CB=CB, cb_lo=cb_lo, bsz=bsz,
                rows_q=rows_q, roff_q=roff_q, goff_q=goff_q, view_lo=view_lo,
                T1=T1, tiles1=tiles1, T2=T2, tiles2=tiles2,
                rbase=rbase, rlen=rlen,
                chunk_slot_lo=chunk_slot_lo, perms=perms,
                pad1=float(pad1), pad2=float(pad2))
    return in_maps, meta


def host_consts(W0, b0, W1, b1, gamma0, beta0, mean0, var0,
                gamma1, beta1, mean1, var1, act_params):
    vecs = np.concatenate([b0, gamma0, beta0, mean0, var0,
                           b1, gamma1, beta1, mean1, var1]).astype(np.float32).reshape(1, 1280)
    ident = np.eye(128, dtype=np.float32)
    return {
        "w0": W0.astype(np.float32),
        "w1": W1.astype(np.float32),
        "vecs": vecs,
        "actp": act_params.reshape(1, 2).astype(np.float32),
        "ident": ident,
    }


# ---------------------------------------------------------------- builder

def build(meta, cfg: Cfg):
    NL, NB, NC = meta["NL"], meta["NB"], meta["NC"]
    CB, cb_lo, bsz = meta["CB"], meta["cb_lo"], meta["bsz"]
    rows_q, roff_q, goff_q = meta["rows_q"], meta["roff_q"], meta["goff_q"]
    view_lo = meta["view_lo"]
    T1, tiles1 = meta["T1"], meta["tiles1"]
    T2 = meta["T2"]
    rbase, rlen = meta["rbase"], meta["rlen"]
    chunk_slot_lo = meta["chunk_slot_lo"]
    N, P, GM, GS = cfg.N, cfg.P, cfg.GM, cfg.GS

    nc = bacc.Bacc(None, target_bir_lowering=False, debug=False)

    xe_ext = nc.declare_dram_parameter("xe", [128, T1 * 128], BF16, isOutput=False)
    mew1_ext = nc.declare_dram_parameter("mew1", [128, T1 * 128], BF16, isOutput=False)
    mew2_ext = nc.declare_dram_parameter("mew2", [128, T2 * 128], BF16, isOutput=False)
    idx_ext = nc.declare_dram_parameter("idx", [128, 8 * T2], I16, isOutput=False)
    wself_ext = nc.declare_dram_parameter("wself", [128, NB], F32, isOutput=False)
    w0_ext = nc.declare_dram_parameter("w0", [128, 128], F32, isOutput=False)
    w1_ext = nc.declare_dram_parameter("w1", [128, 128], F32, isOutput=False)
    vecs_ext = nc.declare_dram_parameter("vecs", [1, 1280], F32, isOutput=False)
    actp_ext = nc.declare_dram_parameter("actp", [1, 2], F32, isOutput=False)
    ident_ext = nc.declare_dram_parameter("ident", [128, 128], F32, isOutput=False)
    out_ext = nc.declare_dram_parameter("out", [NL, 128], F32, isOutput=True)

    hs2_loc = nc.dram_tensor("hs2_loc", [NL, 128], BF16)
    warm_in = nc.dram_tensor("warm_in", [1, 128], BF16)
    warm_out = nc.dram_tensor("warm_out", [P, 128], BF16, addr_space="Shared")
    hs2_full = nc.dram_tensor("hs2_full", [N, 128], BF16, addr_space="Shared")
    groups = [list(range(P))]

    with tile.TileContext(nc, num_cores=P) as tc, ExitStack() as ctx:
        nc.gpsimd.load_library(library_config.mlp)
        # tiny dep-free collective: absorbs the first-collective warmup and
        # trigger latency behind the NRT barrier, so AG-0 runs at full rate
        nc.gpsimd.collective_compute(
            "AllGather", OP.bypass, replica_groups=[list(range(P))],
            ins=[warm_in[:, :]], outs=[warm_out[:, :]])
        cst = ctx.enter_context(tc.tile_pool(name="cst", bufs=1))
        w0_sb = cst.tile([128, 128], F32)
        w1_sb = cst.tile([128, 128], F32)
        w0p = cst.tile([128, 128], BF16)
        w1p = cst.tile([128, 128], BF16)
        vecs_sb = cst.tile([1, 1280], F32)
        actp_sb = cst.tile([1, 2], F32)
        ident_sb = cst.tile([128, 128], F32)
        identb = cst.tile([128, 128], BF16)
        ones_row = cst.tile([1, 128], F32)
        idx_sb = cst.tile([128, 8 * T2], I16)
        wself_sb = cst.tile([128, NB], F32)
        alpha_col = cst.tile([128, 1], F32)
        nalpha_col = cst.tile([128, 1], F32)
        c0_col = cst.tile([128, 1], F32)
        c0a_col = cst.tile([128, 1], F32)
        s0_rep = cst.tile([128, 128], F32)
        s1_rep = cst.tile([128, 128], F32)
        c1_rep = cst.tile([128, 128], F32)
        out_acc = cst.tile([128, NB * 128], F32)
        scratch = cst.tile([1, 6 * 128], F32)

        nc.sync.dma_start(out=w0_sb[:, :], in_=w0_ext[:, :])
        nc.sync.dma_start(out=w1_sb[:, :], in_=w1_ext[:, :])
        nc.sync.dma_start(out=vecs_sb[:, :], in_=vecs_ext[:, :])
        nc.sync.dma_start(out=actp_sb[:, :], in_=actp_ext[:, :])
        nc.sync.dma_start(out=ident_sb[:, :], in_=ident_ext[:, :])
        nc.sync.dma_start(out=wself_sb[:, :], in_=wself_ext[:, :])
        nc.vector.memset(ones_row[:, :], 1.0)
        nc.vector.tensor_copy(identb[:, :], ident_sb[:, :])
        nc.vector.memset(out_acc[:, :], 0.0)

        # ---------------- BN folds (rows in scratch)
        def vrow(i):
            return vecs_sb[0:1, i * 128:(i + 1) * 128]
        s0 = scratch[0:1, 0:128]; c0 = scratch[0:1, 128:256]
        s1 = scratch[0:1, 256:384]; c1 = scratch[0:1, 384:512]
        tmp = scratch[0:1, 512:640]
        nc.vector.tensor_scalar_add(tmp, vrow(4), cfg.bn_eps)
        nc.scalar.activation(s0, tmp, AF.Sqrt)
        nc.vector.reciprocal(s0, s0)
        nc.vector.tensor_mul(s0, s0, vrow(1))
        nc.vector.tensor_sub(tmp, vrow(0), vrow(3))
        nc.vector.tensor_mul(tmp, tmp, s0)
        nc.vector.tensor_add(c0, tmp, vrow(2))
        nc.vector.tensor_scalar_add(tmp, vrow(9), cfg.bn_eps)
        nc.scalar.activation(s1, tmp, AF.Sqrt)
        nc.vector.reciprocal(s1, s1)
        nc.vector.tensor_mul(s1, s1, vrow(6))
        nc.vector.tensor_sub(tmp, vrow(5), vrow(8))
        nc.vector.tensor_mul(tmp, tmp, s1)
        nc.vector.tensor_add(c1, tmp, vrow(7))

        alpha11 = scratch[0:1, 640:641]
        nc.scalar.activation(alpha11, actp_sb[0:1, 0:1], AF.Sigmoid)
        ps_ag = ctx.enter_context(tc.tile_pool(name="ps_ag", bufs=2, space="PSUM"))
        ps_o = ctx.enter_context(tc.tile_pool(name="ps_o", bufs=2, space="PSUM"))
        # row -> broadcast [128,128] (value along free dim)
        for row, rep in ((s0, s0_rep), (s1, s1_rep), (c1, c1_rep)):
            pr = ps_ag.tile([128, 128], F32, tag="ag")
            nc.tensor.matmul(pr[:, :], ones_row[:, :], row)
            nc.scalar.activation(rep[:, :], pr[:, :], AF.Copy)
        # c0 as a column (value along partitions): stationary=c0 row, moving=1x1
        pc = ps_ag.tile([128, 128], F32, tag="ag")
        nc.tensor.matmul(pc[:, 0:1], c0, ones_row[0:1, 0:1])
        nc.scalar.activation(c0_col[:, :], pc[:, 0:1], AF.Copy)
        pa = ps_ag.tile([128, 128], F32, tag="ag")
        nc.tensor.matmul(pa[:, 0:1], ones_row[:, :], alpha11)
        nc.scalar.activation(alpha_col[:, :], pa[:, 0:1], AF.Copy)
        nc.vector.tensor_scalar(nalpha_col[:, :], alpha_col[:, :], -1.0, 1.0,
                                OP.mult, OP.add)
        nc.vector.tensor_mul(c0a_col[:, :], c0_col[:, :], alpha_col[:, :])
        # fold BN scale into weights (bf16 copies)
        nc.vector.tensor_mul(w0p[:, :], w0_sb[:, :], s0_rep[:, :])
        nc.vector.tensor_mul(w1p[:, :], w1_sb[:, :], s1_rep[:, :])

        # ---------------- generic slot-stream chunk helper
        def make_chunk(ext, pool, tag, width, dt, total):
            cache = {}

            def get(sl):
                ch = sl // GM
                if ch not in cache:
                    lo = ch * GM
                    hi = min(total, lo + GM)
                    t_ = pool.tile([128, GM * width], dt, tag=tag)
                    nc.sync.dma_start(out=t_[:, 0:(hi - lo) * width],
                                      in_=ext[:, lo * width:hi * width])
                    cache.clear()
                    cache[ch] = (t_, lo)
                t_, lo = cache[ch]
                return t_[:, (sl - lo) * width:(sl - lo + 1) * width]
            return get

        xep = ctx.enter_context(tc.tile_pool(name="xep", bufs=3))
        m1p = ctx.enter_context(tc.tile_pool(name="m1p", bufs=3))
        m2p = ctx.enter_context(tc.tile_pool(name="m2p", bufs=5))
        gpool = ctx.enter_context(tc.tile_pool(name="gpool", bufs=20))
        wk = ctx.enter_context(tc.tile_pool(name="wk", bufs=3))
        psm = ctx.enter_context(tc.tile_pool(name="psm", bufs=4, space="PSUM"))

        xe_chunk = make_chunk(xe_ext, xep, "xe", 128, BF16, T1)
        m1_chunk = make_chunk(mew1_ext, m1p, "m1", 128, BF16, T1)
        m2_chunk = make_chunk(mew2_ext, m2p, "m2", 128, BF16, T2)

        # ---------------- pass-1 + table for one chunk of blocks
        si1 = [0]

        def pass1_chunk(q):
            for b in range(cb_lo[q], cb_lo[q + 1]):
                pass1_block(b)

        def pass1_block(b):
                nsl = int(tiles1[b])
                col = slice(b * 128, (b + 1) * 128)
                ag = ps_ag.tile([128, 128], F32, tag="ag")
                for j in range(nsl):
                    sl = si1[0] + j
                    nc.tensor.matmul(ag[:, :], xe_chunk(sl), m1_chunk(sl),
                                     start=(j == 0), stop=(j == nsl - 1))
                si1[0] += nsl
                agb = wk.tile([128, 128], BF16, tag="agb")
                nc.vector.tensor_copy(agb[:, :], ag[:, :])
                # o [feat, dst] = (W0*s0)^T-free orientation: psum[i=f,j=d]
                o_ps = ps_o.tile([128, 128], F32, tag="o")
                nc.tensor.matmul(o_ps[:, :], w0p[:, :], agb[:, :], start=True, stop=True)
                r = wk.tile([128, 128], F32, tag="r")
                g = wk.tile([128, 128], F32, tag="g")
                # r = alpha*relu(o+c0) = relu(alpha*o + alpha*c0)
                nc.scalar.activation(r[:, :], o_ps[:, :], AF.Relu,
                                     bias=c0a_col[:, 0:1], scale=alpha_col[:, 0:1])
                nc.scalar.activation(g[:, :], o_ps[:, :], AF.Gelu,
                                     bias=c0_col[:, 0:1])
                g2 = wk.tile([128, 128], F32, tag="g2")
                nc.vector.tensor_scalar(g2[:, :], g[:, :], nalpha_col[:, 0:1],
                                        None, OP.mult)
                y1b = wk.tile([128, 128], BF16, tag="y1b")
                nc.vector.tensor_add(y1b[:, :], r[:, :], g2[:, :])
                # table: h2 [f2, d] = (W1*s1)^T ... ; transpose to [d, f2]
                h2 = ps_o.tile([128, 128], F32, tag="o")
                nc.tensor.matmul(h2[:, :], w1p[:, :], y1b[:, :], start=True, stop=True)
                h2sb = wk.tile([128, 128], BF16, tag="h2sb")
                nc.scalar.activation(h2sb[:, :], h2[:, :], AF.Copy)
                pt = ps_ag.tile([128, 128], F32, tag="ag")
                nc.tensor.matmul(pt[:, :], h2sb[:, :], identb[:, :], start=True, stop=True)
                st = wk.tile([128, 128], BF16, tag="st")
                nc.scalar.activation(st[:, :], pt[:, :], AF.Copy)
                rows = int(bsz[b])
                nc.sync.dma_start(out=hs2_loc[b * 128:b * 128 + rows, :],
                                  in_=st[0:rows, :])
                # self-loop + c1 bias: out_acc[b] += pt*dinv^2 + c1 (adds commute)
                sl_t = wk.tile([128, 128], F32, tag="slt")
                nc.vector.tensor_scalar(sl_t[:, :], pt[:, :], wself_sb[:, b:b + 1],
                                        None, OP.mult)
                nc.vector.tensor_add(sl_t[:, :], sl_t[:, :], c1_rep[:, :])
                nc.vector.tensor_add(out_acc[:, col], out_acc[:, col], sl_t[:, :])

        def allgather_chunk(q):
            nc.gpsimd.collective_compute(
                "AllGather", OP.bypass, replica_groups=groups,
                ins=[hs2_loc[roff_q[q]:roff_q[q + 1], :]],
                outs=[hs2_full[goff_q[q]:goff_q[q + 1], :]])

        # ---------------- pass-2 gather+mew for one chunk
        g_tiles = {}

        def g_slot(q, sl):
            # sl is the global stream slot; calls grouped within phase q; the
            # gather window is the two-chunk span [view_lo[q], goff_q[q+1]).
            lo_q, hi_q = int(chunk_slot_lo[q]), int(chunk_slot_lo[q + 1])
            ch = (sl - lo_q) // GS
            key = (q, ch)
            if key not in g_tiles:
                lo = lo_q + ch * GS
                hi = min(hi_q, lo + GS)
                S = hi - lo
                t_ = gpool.tile([128, GS, 128], BF16, tag="gt")
                nc.gpsimd.dma_gather(
                    t_[:, 0:S, :], hs2_full[int(view_lo[q]):int(goff_q[q + 1]), :],
                    idx_sb[:, lo * 8:hi * 8],
                    num_idxs=S * 128, num_idxs_reg=S * 128, elem_size=128)
                g_tiles.clear()
                g_tiles[key] = (t_, lo)
            t_, lo = g_tiles[key]
            return t_[:, sl - lo, :]

        def gather_chunk(q, extra_cb=None, p1_list=()):
            # emit per-block psum accumulations; pass-1 blocks of the NEXT
            # chunk are injected between mew groups (spread over the first
            # ~55% of the phase) so PE serves both streams continuously; the
            # AG trigger (extra_cb) fires after the last injected block.
            lo_q, hi_q = int(chunk_slot_lo[q]), int(chunk_slot_lo[q + 1])
            p1_list = list(p1_list)
            n_inj = len(p1_list)
            span = (hi_q - lo_q) * 0.40
            inj_at = [lo_q + int(span * (i + 1) / max(n_inj, 1)) for i in range(n_inj)]
            frac = cfg.trig_fracs[min(q, len(cfg.trig_fracs) - 1)]
            trig_at = lo_q + int((hi_q - lo_q) * (0.65 if n_inj else frac))
            fired = [extra_cb is None]
            inj_i = [0]
            for b in range(NB):
                nsl = int(rlen[q, b])
                if nsl == 0:
                    continue
                col = slice(b * 128, (b + 1) * 128)
                pm = psm.tile([128, 128], F32, tag="pm")
                for j in range(nsl):
                    sl = int(rbase[q, b]) + j
                    while inj_i[0] < n_inj and sl >= inj_at[inj_i[0]]:
                        pass1_block(p1_list[inj_i[0]])
                        inj_i[0] += 1
                    if not fired[0] and sl >= trig_at and inj_i[0] == n_inj:
                        extra_cb()
                        fired[0] = True
                    nc.tensor.matmul(pm[:, :], m2_chunk(sl), g_slot(q, sl),
                                     start=(j == 0), stop=(j == nsl - 1))
                nc.vector.tensor_add(out_acc[:, col], out_acc[:, col], pm[:, :])
                if q == int(last_phase[b]):
                    rows = int(bsz[b])
                    nc.sync.dma_start(out=out_ext[b * 128:b * 128 + rows, :],
                                      in_=out_acc[0:rows, col])
            while inj_i[0] < n_inj:
                pass1_block(p1_list[inj_i[0]])
                inj_i[0] += 1
            if not fired[0]:
                extra_cb()

        # last phase with content per block (store emitted right after it)
        last_phase = np.zeros(NB, np.int64)
        for b in range(NB):
            nz = [q for q in range(NC) if rlen[q, b] > 0]
            last_phase[b] = nz[-1] if nz else -1

        # ---------------- emission: chunk-pipelined
        # PE warmup: ~5us of back-to-back matmuls releases the HAM clock gate
        # (1.2 -> 2.4 GHz) before chunk-0's real matmuls issue.
        wm = ps_o.tile([128, 128], F32, tag="o")
        for _ in range(48):
            nc.tensor.matmul(wm[:, :], ident_sb[:, :], ident_sb[:, :],
                             start=True, stop=False)
        nc.tensor.matmul(wm[:, :], ident_sb[:, :], ident_sb[:, :],
                         start=False, stop=True)

        pass1_chunk(0)
        # idx load off the startup critical path (only gathers consume it)
        nc.sync.dma_start(out=idx_sb[:, :], in_=idx_ext[:, :])
        allgather_chunk(0)
        for q in range(NC):
            if q + 1 < NC:
                pass1_chunk(q + 1)
                gather_chunk(q, extra_cb=lambda qq=q: allgather_chunk(qq + 1))
            else:
                gather_chunk(q)
        # blocks with no pass-2 edges at all (fully padded): store after init
        for b in range(NB):
            if last_phase[b] < 0:
                rows = int(bsz[b])
                nc.sync.dma_start(out=out_ext[b * 128:b * 128 + rows, :],
                                  in_=out_acc[0:rows, b * 128:b * 128 + 128])

    nc.finalize()
    return nc


# ---------------------------------------------------------------- runners

def prep_all(inputs, cfg: Cfg):
    in_maps, meta = host_prep(inputs["x"], inputs["edge_index"],
                              inputs["edge_weight"], cfg)
    consts = host_consts(inputs["W0"], inputs["b0"], inputs["W1"], inputs["b1"],
                         inputs["gamma0"], inputs["beta0"], inputs["mean0"],
                         inputs["var0"], inputs["gamma1"], inputs["beta1"],
                         inputs["mean1"], inputs["var1"], inputs["act_params"])
    for m in in_maps:
        m.update(consts)
    return in_maps, meta


def unshard(results, cfg: Cfg, meta=None):
    NL = cfg.N // cfg.P
    out = np.zeros((cfg.N, cfg.D), np.float32)
    for c in range(cfg.P):
        r = results[c]["out"]
        if meta is not None and "perms" in meta:
            out[c * NL:(c + 1) * NL] = r[meta["perms"][c]]
        else:
            out[c * NL:(c + 1) * NL] = r
    return out


# ---------------------------------------------------------------- entrypoint

def _install_dge_patch():
    """walrus needs --dge-levels=vector_dynamic_offsets for indirect DMAs."""
    from concourse import bass_utils as _bu
    if getattr(_bu, "_gcn_dge_patched", False):
        return
    _orig = _bu.run_command

    def _patched(argv, **kwargs):
        if argv and "walrus_driver" in str(argv[0]) and not any(
                str(a).startswith("--dge-levels") for a in argv):
            argv = list(argv) + ["--dge-levels=vector_dynamic_offsets"]
        return _orig(argv, **kwargs)

    _bu.run_command = _patched
    _bu._gcn_dge_patched = True


_CFG = Cfg()


def kernel(**inputs):
    """Full-input entrypoint: shard, run on 8 NeuronCores, gather output."""
    import numpy as np
    _install_dge_patch()
    inputs = {k: np.asarray(v) for k, v in inputs.items()}
    in_maps, meta = prep_all(inputs, _CFG)
    nc = build(meta, _CFG)
    res = run_bass_kernel_spmd(nc, in_maps, core_ids=list(range(_CFG.P)))
    return unshard([{k: np.asarray(v) for k, v in r.items()} for r in res.results],
                   _CFG, meta)


# revision 50
# speedup vs baseline: 1.0880x; 1.0880x over previous
"""AdaptiveGCN (2-layer GCNConv + BN eval + adaptive relu/gelu blend) on 8 TRN2 cores.

v4 strategy (chunk-pipelined gather):
  The hard floor on TRN2 is the pass-2 device gather: every in-edge needs one
  SWDGE descriptor pair (~8.4ns/edge of GpSimd ucode time), so the per-core
  gather of E/8 = 75k table rows costs ~650us no matter how it's expressed
  (dma_gather / indirect_dma_start / ap_gather all measured at or above this
  rate). v4 therefore hides *everything else* under the gather stream:

  - Nodes are dst-sharded; each core's 49 dst blocks are grouped into 4
    fixed chunks (quarters of the node range, permutation-independent).
  - Pass 1 (conv1+BN+blend) runs chunk by chunk; each chunk's table slice
    (h2 = y1 @ W1*s1, transposed per block on PE) is stored and AllGathered
    immediately, so pass-2 gathers of chunk 0 start ~55us into the kernel
    while pass 1 of chunks 1-3 still runs on PE/DVE/ACT.
  - Pass-2 accumulation is chunk-major: per (chunk, block) a short psum
    accumulation over edge slot tiles (one-hot mew matmuls), then a DVE add
    into a persistent f32 out_acc region (adds commute - no ordering needed).
  - Self-loops never enter the gather: at table-build time the block's own
    (transposed) table psum is scaled by dinv^2 and added into out_acc.
  - deg/dinv are host-precomputed from (edge_index, edge_weight) and folded
    into the streamed one-hot weights (mew1/mew2), killing the v3 dsw/dgl
    streams and the per-chunk DVE broadcast scaling.
  - BN folds: scale into W0/W1 (bf16), bias via scalar-engine activation
    (bias/scale per partition; psum orientation is [feat, dst] in pass 1).
  - AllGather is split into 4 sub-collectives; each trigger is emitted
    between gather calls of the previous chunk so GpSimd never stalls.
"""

import dataclasses
import ml_dtypes
import numpy as np
from contextlib import ExitStack

from concourse import bass, bacc, mybir, tile, library_config
from concourse.bass_utils import run_bass_kernel_spmd

F32 = mybir.dt.float32
BF16 = mybir.dt.bfloat16
I16 = mybir.dt.int16
I32 = mybir.dt.int32
AF = mybir.ActivationFunctionType
OP = mybir.AluOpType
AX = mybir.AxisListType


@dataclasses.dataclass
class Cfg:
    N: int = 50000
    E: int = 600000
    D: int = 128
    P: int = 8            # cores
    BLK: int = 128        # dst nodes per block
    GM: int = 32          # stream slots per chunk-load
    GS: int = 8           # slots per dma_gather (1024-idx ucode cap)
    bn_eps: float = 1e-5
    chunk_blocks: tuple = (9, 12, 13, 15)    # dst-block counts per pipeline chunk
    trig_fracs: tuple = (0.25, 0.25, 0.25)  # AG q+1 trigger position in phase q


# ---------------------------------------------------------------- host prep

def host_prep(x, edge_index, edge_weight, cfg: Cfg):
    N, E, P, BLK = cfg.N, cfg.E, cfg.P, cfg.BLK
    NL = N // P
    NB = (NL + BLK - 1) // BLK
    CB = list(cfg.chunk_blocks)
    NC = len(CB)
    assert sum(CB) == NB
    # block sizes (last block is short)
    bsz = np.full(NB, BLK, np.int64)
    bsz[NB - 1] = NL - BLK * (NB - 1)
    cb_lo = np.concatenate([[0], np.cumsum(CB)]).astype(np.int64)   # chunk -> first block
    # rows per chunk (local), chunk row offsets (local)
    rows_q = np.array([bsz[cb_lo[q]:cb_lo[q + 1]].sum() for q in range(NC)], np.int64)
    roff_q = np.concatenate([[0], np.cumsum(rows_q)]).astype(np.int64)
    # global table layout: [AG0: P*rows_0 | AG1: P*rows_1 | ...]
    goff_q = np.concatenate([[0], np.cumsum(P * rows_q)]).astype(np.int64)
    assert all(P * rows_q <= 32767)

    srcE = edge_index[0].astype(np.int64)
    dstE = edge_index[1].astype(np.int64)
    ewE = edge_weight.astype(np.float64)

    # host deg/dinv (pure input math; self-loop weight 1)
    deg = np.bincount(dstE, weights=ewE, minlength=N) + 1.0
    dinv = 1.0 / np.sqrt(deg)

    # chunk of a node: by its *natural* local position (fixed, perm-independent)
    loc_all = np.arange(N) % NL
    chunk_of = np.searchsorted(roff_q[1:], loc_all, side="right")  # [N] in [0, NC)

    # ---- per-core greedy: assign local nodes to blocks inside their chunk,
    # balancing the NC per-chunk in-edge counts + total (incl self-loop).
    core_d = dstE // NL
    kcnt = np.zeros((N, NC), np.float64)  # in-edges of node d from src-chunk k
    np.add.at(kcnt, (dstE, chunk_of[srcE]), 1.0)

    perms = []
    for c in range(P):
        pnew = np.zeros(NL, np.int64)
        for q in range(NC):
            nlo, nhi = roff_q[q], roff_q[q + 1]
            nodes = np.arange(nlo, nhi)              # local ids in this chunk
            kc = kcnt[c * NL + nodes]                # [nq, NC]
            tot = kc.sum(axis=1) + 1.0
            nb = CB[q]
            caps = bsz[cb_lo[q]:cb_lo[q + 1]].astype(np.int64)
            # per-(block,chunk) quotas: stagger tile counts so most cells sit
            # just under a 128 boundary (26-edge margin for cross-core jitter)
            dem = kc.sum(axis=0)                     # per-chunk demand in this quarter
            quota = np.zeros((nb, NC))
            for k in range(NC):
                m = dem[k] / nb
                t_hi = int(np.ceil((m + 26.0) / 128.0))
                cap_hi = t_hi * 128.0 - 26.0
                # how many blocks can drop one tile while still fitting
                nshrink = 0
                if t_hi > 1:
                    cap_lo = (t_hi - 1) * 128.0 - 26.0
                    nshrink = int(min(nb - 1, max(0.0, (nb * cap_hi - dem[k] * 1.03) // (cap_hi - cap_lo))))
                col = np.full(nb, cap_hi)
                col[:nshrink] = (t_hi - 1) * 128.0 - 26.0
                quota[:, k] = np.roll(col, k)        # stagger across chunks
            tq = (tot.sum() + nb - 1) / nb + 20.0
            order_n = np.argsort(-tot, kind="stable")
            fill = np.zeros(nb, np.int64)
            L = np.zeros((nb, NC)); Tt = np.zeros(nb)
            for n in order_n:
                cost = np.max((L + kc[n]) / quota, axis=1)
                cost = np.maximum(cost, (Tt + tot[n]) / (13.0 * 128.0))
                cost = np.maximum(cost, (Tt + tot[n]) / tq * 0.01)
                cost[fill >= caps] = 1e9
                bi = int(np.argmin(cost))
                pnew[nodes[n]] = (cb_lo[q] + bi) * BLK + fill[bi]
                fill[bi] += 1
                L[bi] += kc[n]; Tt[bi] += tot[n]
        perms.append(pnew)

    # global table row of node n: chunk q region + core stripe + chunk offset
    tpos_g = np.zeros(N, np.int64)
    for c in range(P):
        pn = perms[c]                    # local node -> new local pos (block*128+off, block-major)
        q = chunk_of[c * NL + np.arange(NL)]
        posq = pn - cb_lo[q] * BLK       # position within own chunk (block-major)
        tpos_g[c * NL:(c + 1) * NL] = goff_q[q] + c * rows_q[q] + posq
    x_bf = np.asarray(x, np.float32).astype(ml_dtypes.bfloat16)
    dinv32 = dinv.astype(np.float32)

    # ---------------- pass-1 schedule: edges + self-loops, sorted (block, dst)
    wl1 = (dinv[srcE] * ewE * dinv[dstE]).astype(np.float32)
    src1 = np.concatenate([srcE, np.arange(N, dtype=np.int64)])
    dst1 = np.concatenate([dstE, np.arange(N, dtype=np.int64)])
    # self-loop pass-1 weight would be dinv^2 but self term is added at table
    # time in pass 2 only; pass 1 needs it in the aggregation: conv1 includes
    # self-loops, so keep them in the pass-1 stream.
    w1 = np.concatenate([wl1, (dinv * dinv).astype(np.float32)])
    core1 = dst1 // NL
    per1, counts1 = [], np.zeros((P, NB), np.int64)
    for c in range(P):
        m = core1 == c
        s_, d_, w_ = src1[m], perms[c][dst1[m] - c * NL], w1[m]
        b_ = d_ // BLK
        o = np.lexsort((d_, b_))
        per1.append((s_[o], d_[o], w_[o], b_[o]))
        counts1[c] = np.bincount(b_, minlength=NB)
    tiles1 = np.ceil(counts1.max(axis=0) / 128).astype(np.int64)
    T1 = int(tiles1.sum())
    sbase1 = np.concatenate([[0], np.cumsum(tiles1)]).astype(np.int64)
    pad1 = (T1 * 128 * P - counts1.sum()) / counts1.sum()

    # ---------------- pass-2 schedule: real edges, dense per-block tiling.
    # Edges sorted (block, src-chunk, dst) and packed densely into each
    # block's lanes; a 128-lane tile may straddle a chunk boundary. Each tile
    # gets a COMMON phase = max src-chunk present in it across cores, and is
    # gathered from a two-chunk table window [goff[p-1], goff[p+1]) so the
    # straddle costs no padding. Slot order is (phase, block, tile).
    w2 = (dinv[srcE] * ewE * dinv[dstE]).astype(np.float32)
    k2 = chunk_of[srcE]
    per2 = []
    counts2 = np.zeros((P, NB), np.int64)            # per-block totals
    cum2 = np.zeros((P, NB, NC + 1), np.int64)       # cumulative per chunk
    for c in range(P):
        m = core_d == c
        s_, d_, w_, k_ = tpos_g[srcE[m]], perms[c][dstE[m] - c * NL], w2[m], k2[m]
        b_ = d_ // BLK
        o = np.lexsort((d_, k_, b_))
        s_, d_, w_, k_, b_ = s_[o], d_[o], w_[o], k_[o], b_[o]
        per2.append((s_, d_, w_, k_, b_))
        counts2[c] = np.bincount(b_, minlength=NB)
        for q in range(NC):
            cum2[c, :, q + 1] = cum2[c, :, q] + np.bincount(b_[k_ == q], minlength=NB)
    tiles2 = np.ceil(counts2.max(axis=0) / 128).astype(np.int64)  # [NB]
    T2 = int(tiles2.sum())
    pad2 = (T2 * 128 * P - counts2.sum()) / counts2.sum()

    # phase of each tile: smallest p s.t. cross-core max cum through chunk p
    # covers the tile start... practically: max over cores of the chunk of the
    # last lane the core has in this tile (cores without content contribute 0)
    phase_b = []                                      # [NB] -> array [tiles_b]
    for b in range(NB):
        tb = int(tiles2[b])
        ph = np.zeros(tb, np.int64)
        for t in range(tb):
            hi_lane = (t + 1) * 128 - 1
            p = 0
            pmin = NC
            for c in range(P):
                tot = counts2[c, b]
                if tot <= t * 128:
                    continue
                last = min(hi_lane, tot - 1)
                pc = int(np.searchsorted(cum2[c, b, 1:], last, side="right"))
                p = max(p, pc)
                pc0 = int(np.searchsorted(cum2[c, b, 1:], t * 128, side="right"))
                pmin = min(pmin, pc0)
            ph[t] = p
            if pmin < NC and pmin < p - 1:
                raise RuntimeError(f"tile spans >2 chunks: b={b} t={t} {pmin} {p}")
        phase_b.append(ph)
    # slot order (phase, block, tile): per (phase, block) runs
    rbase = np.full((NC, NB), -1, np.int64)
    rlen = np.zeros((NC, NB), np.int64)
    slot_of = [np.zeros(int(tiles2[b]), np.int64) for b in range(NB)]
    acc = 0
    chunk_slot_lo = np.zeros(NC + 1, np.int64)
    for q in range(NC):
        chunk_slot_lo[q] = acc
        for b in range(NB):
            ts = np.nonzero(phase_b[b] == q)[0]
            if len(ts) == 0:
                continue
            rbase[q, b] = acc
            rlen[q, b] = len(ts)
            for t in ts:
                slot_of[b][t] = acc
                acc += 1
    chunk_slot_lo[NC] = acc
    assert acc == T2
    tile_base = np.concatenate([[0], np.cumsum(tiles2)]).astype(np.int64)
    slot_flat = np.concatenate(slot_of) if T2 else np.zeros(0, np.int64)
    phase_flat = np.concatenate(phase_b) if T2 else np.zeros(0, np.int64)
    # gather view per phase: [view_lo[q], goff_q[q+1])
    view_lo = goff_q[np.maximum(np.arange(NC) - 1, 0)]

    in_maps = []
    for c in range(P):
        # pass 1 arrays
        s_, d_, w_, b_ = per1[c]
        bs = np.concatenate([[0], np.cumsum(counts1[c])]).astype(np.int64)
        p_ = np.arange(len(b_)) - bs[b_]
        lane, sl = p_ % 128, sbase1[b_] + p_ // 128
        xe = np.zeros((128, T1, 128), ml_dtypes.bfloat16)
        xe[lane, sl, :] = x_bf[s_]
        mew1 = np.zeros((128, T1, 128), ml_dtypes.bfloat16)
        mew1[lane, sl, d_ % BLK] = w_.astype(ml_dtypes.bfloat16)

        # pass 2 arrays (dense per-block lanes, phase-major slots)
        s_, d_, w_, k_, b_ = per2[c]
        bs2 = np.concatenate([[0], np.cumsum(counts2[c])]).astype(np.int64)
        p_ = np.arange(len(b_)) - bs2[b_]
        lane = p_ % 128
        t_ = p_ // 128
        sl = slot_flat[tile_base[b_] + t_]
        ph = phase_flat[tile_base[b_] + t_]
        mew2 = np.zeros((128, T2, 128), ml_dtypes.bfloat16)
        mew2[lane, sl, d_ % BLK] = w_.astype(ml_dtypes.bfloat16)
        idx = np.zeros((16, 8 * T2), np.int16)
        idxval = s_ - view_lo[ph]
        assert idxval.min() >= 0 and idxval.max() <= 32767
        idx[lane % 16, sl * 8 + lane // 16] = idxval.astype(np.int16)

        # self-loop weights dinv^2 in (block, pos) layout
        wself = np.zeros((128, NB), np.float32)
        npos = np.arange(NL)
        newp = perms[c][npos]
        dv = dinv32[c * NL + npos]
        wself[newp % BLK, newp // BLK] = dv * dv

        in_maps.append({
            "xe": xe.reshape(128, T1 * 128),
            "mew1": mew1.reshape(128, T1 * 128),
            "mew2": mew2.reshape(128, T2 * 128),
            "idx": np.tile(idx, (8, 1)),
            "wself": wself,
        })

    meta = dict(NL=NL, NB=NB, NC=NC, CB=CB, cb_lo=cb_lo, bsz=bsz,
                rows_q=rows_q, roff_q=roff_q, goff_q=goff_q, view_lo=view_lo,
                T1=T1, tiles1=tiles1, T2=T2, tiles2=tiles2,
                rbase=rbase, rlen=rlen,
                chunk_slot_lo=chunk_slot_lo, perms=perms,
                pad1=float(pad1), pad2=float(pad2))
    return in_maps, meta


def host_consts(W0, b0, W1, b1, gamma0, beta0, mean0, var0,
                gamma1, beta1, mean1, var1, act_params):
    vecs = np.concatenate([b0, gamma0, beta0, mean0, var0,
                           b1, gamma1, beta1, mean1, var1]).astype(np.float32).reshape(1, 1280)
    ident = np.eye(128, dtype=np.float32)
    return {
        "w0": W0.astype(np.float32),
        "w1": W1.astype(np.float32),
        "vecs": vecs,
        "actp": act_params.reshape(1, 2).astype(np.float32),
        "ident": ident,
    }


# ---------------------------------------------------------------- builder

def build(meta, cfg: Cfg):
    NL, NB, NC = meta["NL"], meta["NB"], meta["NC"]
    CB, cb_lo, bsz = meta["CB"], meta["cb_lo"], meta["bsz"]
    rows_q, roff_q, goff_q = meta["rows_q"], meta["roff_q"], meta["goff_q"]
    view_lo = meta["view_lo"]
    T1, tiles1 = meta["T1"], meta["tiles1"]
    T2 = meta["T2"]
    rbase, rlen = meta["rbase"], meta["rlen"]
    chunk_slot_lo = meta["chunk_slot_lo"]
    N, P, GM, GS = cfg.N, cfg.P, cfg.GM, cfg.GS

    nc = bacc.Bacc(None, target_bir_lowering=False, debug=False)

    xe_ext = nc.declare_dram_parameter("xe", [128, T1 * 128], BF16, isOutput=False)
    mew1_ext = nc.declare_dram_parameter("mew1", [128, T1 * 128], BF16, isOutput=False)
    mew2_ext = nc.declare_dram_parameter("mew2", [128, T2 * 128], BF16, isOutput=False)
    idx_ext = nc.declare_dram_parameter("idx", [128, 8 * T2], I16, isOutput=False)
    wself_ext = nc.declare_dram_parameter("wself", [128, NB], F32, isOutput=False)
    w0_ext = nc.declare_dram_parameter("w0", [128, 128], F32, isOutput=False)
    w1_ext = nc.declare_dram_parameter("w1", [128, 128], F32, isOutput=False)
    vecs_ext = nc.declare_dram_parameter("vecs", [1, 1280], F32, isOutput=False)
    actp_ext = nc.declare_dram_parameter("actp", [1, 2], F32, isOutput=False)
    ident_ext = nc.declare_dram_parameter("ident", [128, 128], F32, isOutput=False)
    out_ext = nc.declare_dram_parameter("out", [NL, 128], F32, isOutput=True)

    hs2_loc = nc.dram_tensor("hs2_loc", [NL, 128], BF16)
    warm_in = nc.dram_tensor("warm_in", [1, 128], BF16)
    warm_out = nc.dram_tensor("warm_out", [P, 128], BF16, addr_space="Shared")
    hs2_full = nc.dram_tensor("hs2_full", [N, 128], BF16, addr_space="Shared")
    groups = [list(range(P))]

    with tile.TileContext(nc, num_cores=P) as tc, ExitStack() as ctx:
        nc.gpsimd.load_library(library_config.mlp)
        # tiny dep-free collective: absorbs the first-collective warmup and
        # trigger latency behind the NRT barrier, so AG-0 runs at full rate
        nc.gpsimd.collective_compute(
            "AllGather", OP.bypass, replica_groups=[list(range(P))],
            ins=[warm_in[:, :]], outs=[warm_out[:, :]])
        cst = ctx.enter_context(tc.tile_pool(name="cst", bufs=1))
        w0_sb = cst.tile([128, 128], F32)
        w1_sb = cst.tile([128, 128], F32)
        w0p = cst.tile([128, 128], BF16)
        w1p = cst.tile([128, 128], BF16)
        vecs_sb = cst.tile([1, 1280], F32)
        actp_sb = cst.tile([1, 2], F32)
        ident_sb = cst.tile([128, 128], F32)
        identb = cst.tile([128, 128], BF16)
        ones_row = cst.tile([1, 128], F32)
        idx_sb = cst.tile([128, 8 * T2], I16)
        wself_sb = cst.tile([128, NB], F32)
        alpha_col = cst.tile([128, 1], F32)
        nalpha_col = cst.tile([128, 1], F32)
        c0_col = cst.tile([128, 1], F32)
        c0a_col = cst.tile([128, 1], F32)
        s0_rep = cst.tile([128, 128], F32)
        s1_rep = cst.tile([128, 128], F32)
        c1_rep = cst.tile([128, 128], F32)
        out_acc = cst.tile([128, NB * 128], F32)
        scratch = cst.tile([1, 6 * 128], F32)

        nc.sync.dma_start(out=w0_sb[:, :], in_=w0_ext[:, :])
        nc.sync.dma_start(out=w1_sb[:, :], in_=w1_ext[:, :])
        nc.sync.dma_start(out=vecs_sb[:, :], in_=vecs_ext[:, :])
        nc.sync.dma_start(out=actp_sb[:, :], in_=actp_ext[:, :])
        nc.sync.dma_start(out=ident_sb[:, :], in_=ident_ext[:, :])
        nc.sync.dma_start(out=wself_sb[:, :], in_=wself_ext[:, :])
        nc.vector.memset(ones_row[:, :], 1.0)
        nc.vector.tensor_copy(identb[:, :], ident_sb[:, :])
        nc.vector.memset(out_acc[:, :], 0.0)

        # ---------------- BN folds (rows in scratch)
        def vrow(i):
            return vecs_sb[0:1, i * 128:(i + 1) * 128]
        s0 = scratch[0:1, 0:128]; c0 = scratch[0:1, 128:256]
        s1 = scratch[0:1, 256:384]; c1 = scratch[0:1, 384:512]
        tmp = scratch[0:1, 512:640]
        nc.vector.tensor_scalar_add(tmp, vrow(4), cfg.bn_eps)
        nc.scalar.activation(s0, tmp, AF.Sqrt)
        nc.vector.reciprocal(s0, s0)
        nc.vector.tensor_mul(s0, s0, vrow(1))
        nc.vector.tensor_sub(tmp, vrow(0), vrow(3))
        nc.vector.tensor_mul(tmp, tmp, s0)
        nc.vector.tensor_add(c0, tmp, vrow(2))
        nc.vector.tensor_scalar_add(tmp, vrow(9), cfg.bn_eps)
        nc.scalar.activation(s1, tmp, AF.Sqrt)
        nc.vector.reciprocal(s1, s1)
        nc.vector.tensor_mul(s1, s1, vrow(6))
        nc.vector.tensor_sub(tmp, vrow(5), vrow(8))
        nc.vector.tensor_mul(tmp, tmp, s1)
        nc.vector.tensor_add(c1, tmp, vrow(7))

        alpha11 = scratch[0:1, 640:641]
        nc.scalar.activation(alpha11, actp_sb[0:1, 0:1], AF.Sigmoid)
        ps_ag = ctx.enter_context(tc.tile_pool(name="ps_ag", bufs=2, space="PSUM"))
        ps_o = ctx.enter_context(tc.tile_pool(name="ps_o", bufs=2, space="PSUM"))
        # row -> broadcast [128,128] (value along free dim)
        for row, rep in ((s0, s0_rep), (s1, s1_rep), (c1, c1_rep)):
            pr = ps_ag.tile([128, 128], F32, tag="ag")
            nc.tensor.matmul(pr[:, :], ones_row[:, :], row)
            nc.scalar.activation(rep[:, :], pr[:, :], AF.Copy)
        # c0 as a column (value along partitions): stationary=c0 row, moving=1x1
        pc = ps_ag.tile([128, 128], F32, tag="ag")
        nc.tensor.matmul(pc[:, 0:1], c0, ones_row[0:1, 0:1])
        nc.scalar.activation(c0_col[:, :], pc[:, 0:1], AF.Copy)
        pa = ps_ag.tile([128, 128], F32, tag="ag")
        nc.tensor.matmul(pa[:, 0:1], ones_row[:, :], alpha11)
        nc.scalar.activation(alpha_col[:, :], pa[:, 0:1], AF.Copy)
        nc.vector.tensor_scalar(nalpha_col[:, :], alpha_col[:, :], -1.0, 1.0,
                                OP.mult, OP.add)
        nc.vector.tensor_mul(c0a_col[:, :], c0_col[:, :], alpha_col[:, :])
        # fold BN scale into weights (bf16 copies)
        nc.vector.tensor_mul(w0p[:, :], w0_sb[:, :], s0_rep[:, :])
        nc.vector.tensor_mul(w1p[:, :], w1_sb[:, :], s1_rep[:, :])

        # ---------------- generic slot-stream chunk helper
        def make_chunk(ext, pool, tag, width, dt, total):
            cache = {}

            def get(sl):
                ch = sl // GM
                if ch not in cache:
                    lo = ch * GM
                    hi = min(total, lo + GM)
                    t_ = pool.tile([128, GM * width], dt, tag=tag)
                    nc.sync.dma_start(out=t_[:, 0:(hi - lo) * width],
                                      in_=ext[:, lo * width:hi * width])
                    cache.clear()
                    cache[ch] = (t_, lo)
                t_, lo = cache[ch]
                return t_[:, (sl - lo) * width:(sl - lo + 1) * width]
            return get

        xep = ctx.enter_context(tc.tile_pool(name="xep", bufs=3))
        m1p = ctx.enter_context(tc.tile_pool(name="m1p", bufs=3))
        m2p = ctx.enter_context(tc.tile_pool(name="m2p", bufs=4))
        gpool = ctx.enter_context(tc.tile_pool(name="gpool", bufs=16))
        wk = ctx.enter_context(tc.tile_pool(name="wk", bufs=3))
        psm = ctx.enter_context(tc.tile_pool(name="psm", bufs=4, space="PSUM"))

        xe_chunk = make_chunk(xe_ext, xep, "xe", 128, BF16, T1)
        m1_chunk = make_chunk(mew1_ext, m1p, "m1", 128, BF16, T1)
        m2_chunk = make_chunk(mew2_ext, m2p, "m2", 128, BF16, T2)

        # ---------------- pass-1 + table for one chunk of blocks
        si1 = [0]

        def pass1_chunk(q):
            for b in range(cb_lo[q], cb_lo[q + 1]):
                pass1_block(b)

        def pass1_block(b):
                nsl = int(tiles1[b])
                col = slice(b * 128, (b + 1) * 128)
                ag = ps_ag.tile([128, 128], F32, tag="ag")
                for j in range(nsl):
                    sl = si1[0] + j
                    nc.tensor.matmul(ag[:, :], xe_chunk(sl), m1_chunk(sl),
                                     start=(j == 0), stop=(j == nsl - 1))
                si1[0] += nsl
                agb = wk.tile([128, 128], BF16, tag="agb")
                nc.vector.tensor_copy(agb[:, :], ag[:, :])
                # o [feat, dst] = (W0*s0)^T-free orientation: psum[i=f,j=d]
                o_ps = ps_o.tile([128, 128], F32, tag="o")
                nc.tensor.matmul(o_ps[:, :], w0p[:, :], agb[:, :], start=True, stop=True)
                r = wk.tile([128, 128], F32, tag="r")
                g = wk.tile([128, 128], F32, tag="g")
                # r = alpha*relu(o+c0) = relu(alpha*o + alpha*c0)
                nc.scalar.activation(r[:, :], o_ps[:, :], AF.Relu,
                                     bias=c0a_col[:, 0:1], scale=alpha_col[:, 0:1])
                nc.scalar.activation(g[:, :], o_ps[:, :], AF.Gelu,
                                     bias=c0_col[:, 0:1])
                g2 = wk.tile([128, 128], F32, tag="g2")
                nc.vector.tensor_scalar(g2[:, :], g[:, :], nalpha_col[:, 0:1],
                                        None, OP.mult)
                y1b = wk.tile([128, 128], BF16, tag="y1b")
                nc.vector.tensor_add(y1b[:, :], r[:, :], g2[:, :])
                # table: h2 [f2, d] = (W1*s1)^T ... ; transpose to [d, f2]
                h2 = ps_o.tile([128, 128], F32, tag="o")
                nc.tensor.matmul(h2[:, :], w1p[:, :], y1b[:, :], start=True, stop=True)
                h2sb = wk.tile([128, 128], BF16, tag="h2sb")
                nc.scalar.activation(h2sb[:, :], h2[:, :], AF.Copy)
                pt = ps_ag.tile([128, 128], F32, tag="ag")
                nc.tensor.matmul(pt[:, :], h2sb[:, :], identb[:, :], start=True, stop=True)
                st = wk.tile([128, 128], BF16, tag="st")
                nc.scalar.activation(st[:, :], pt[:, :], AF.Copy)
                rows = int(bsz[b])
                nc.sync.dma_start(out=hs2_loc[b * 128:b * 128 + rows, :],
                                  in_=st[0:rows, :])
                # self-loop + c1 bias: out_acc[b] += pt*dinv^2 + c1 (adds commute)
                sl_t = wk.tile([128, 128], F32, tag="slt")
                nc.vector.tensor_scalar(sl_t[:, :], pt[:, :], wself_sb[:, b:b + 1],
                                        None, OP.mult)
                nc.vector.tensor_add(sl_t[:, :], sl_t[:, :], c1_rep[:, :])
                nc.vector.tensor_add(out_acc[:, col], out_acc[:, col], sl_t[:, :])

        def allgather_chunk(q):
            nc.gpsimd.collective_compute(
                "AllGather", OP.bypass, replica_groups=groups,
                ins=[hs2_loc[roff_q[q]:roff_q[q + 1], :]],
                outs=[hs2_full[goff_q[q]:goff_q[q + 1], :]])

        # ---------------- pass-2 gather+mew for one chunk
        g_tiles = {}

        def g_slot(q, sl):
            # sl is the global stream slot; calls grouped within phase q; the
            # gather window is the two-chunk span [view_lo[q], goff_q[q+1]).
            lo_q, hi_q = int(chunk_slot_lo[q]), int(chunk_slot_lo[q + 1])
            ch = (sl - lo_q) // GS
            key = (q, ch)
            if key not in g_tiles:
                lo = lo_q + ch * GS
                hi = min(hi_q, lo + GS)
                S = hi - lo
                t_ = gpool.tile([128, GS, 128], BF16, tag="gt")
                nc.gpsimd.dma_gather(
                    t_[:, 0:S, :], hs2_full[int(view_lo[q]):int(goff_q[q + 1]), :],
                    idx_sb[:, lo * 8:hi * 8],
                    num_idxs=S * 128, num_idxs_reg=S * 128, elem_size=128)
                g_tiles.clear()
                g_tiles[key] = (t_, lo)
            t_, lo = g_tiles[key]
            return t_[:, sl - lo, :]

        def gather_chunk(q, extra_cb=None, p1_list=()):
            # emit per-block psum accumulations; pass-1 blocks of the NEXT
            # chunk are injected between mew groups (spread over the first
            # ~55% of the phase) so PE serves both streams continuously; the
            # AG trigger (extra_cb) fires after the last injected block.
            lo_q, hi_q = int(chunk_slot_lo[q]), int(chunk_slot_lo[q + 1])
            p1_list = list(p1_list)
            n_inj = len(p1_list)
            span = (hi_q - lo_q) * 0.40
            inj_at = [lo_q + int(span * (i + 1) / max(n_inj, 1)) for i in range(n_inj)]
            frac = cfg.trig_fracs[min(q, len(cfg.trig_fracs) - 1)]
            trig_at = lo_q + int((hi_q - lo_q) * (0.65 if n_inj else frac))
            fired = [extra_cb is None]
            inj_i = [0]
            for b in range(NB):
                nsl = int(rlen[q, b])
                if nsl == 0:
                    continue
                col = slice(b * 128, (b + 1) * 128)
                pm = psm.tile([128, 128], F32, tag="pm")
                for j in range(nsl):
                    sl = int(rbase[q, b]) + j
                    while inj_i[0] < n_inj and sl >= inj_at[inj_i[0]]:
                        pass1_block(p1_list[inj_i[0]])
                        inj_i[0] += 1
                    if not fired[0] and sl >= trig_at and inj_i[0] == n_inj:
                        extra_cb()
                        fired[0] = True
                    nc.tensor.matmul(pm[:, :], m2_chunk(sl), g_slot(q, sl),
                                     start=(j == 0), stop=(j == nsl - 1))
                nc.vector.tensor_add(out_acc[:, col], out_acc[:, col], pm[:, :])
                if q == int(last_phase[b]):
                    rows = int(bsz[b])
                    nc.sync.dma_start(out=out_ext[b * 128:b * 128 + rows, :],
                                      in_=out_acc[0:rows, col])
            while inj_i[0] < n_inj:
                pass1_block(p1_list[inj_i[0]])
                inj_i[0] += 1
            if not fired[0]:
                extra_cb()

        # last phase with content per block (store emitted right after it)
        last_phase = np.zeros(NB, np.int64)
        for b in range(NB):
            nz = [q for q in range(NC) if rlen[q, b] > 0]
            last_phase[b] = nz[-1] if nz else -1

        # ---------------- emission: chunk-pipelined
        # PE warmup: ~5us of back-to-back matmuls releases the HAM clock gate
        # (1.2 -> 2.4 GHz) before chunk-0's real matmuls issue.
        wm = ps_o.tile([128, 128], F32, tag="o")
        for _ in range(48):
            nc.tensor.matmul(wm[:, :], ident_sb[:, :], ident_sb[:, :],
                             start=True, stop=False)
        nc.tensor.matmul(wm[:, :], ident_sb[:, :], ident_sb[:, :],
                         start=False, stop=True)

        pass1_chunk(0)
        # idx load off the startup critical path (only gathers consume it)
        nc.sync.dma_start(out=idx_sb[:, :], in_=idx_ext[:, :])
        allgather_chunk(0)
        for q in range(NC):
            if q + 1 < NC:
                pass1_chunk(q + 1)
                gather_chunk(q, extra_cb=lambda qq=q: allgather_chunk(qq + 1))
            else:
                gather_chunk(q)
        # blocks with no pass-2 edges at all (fully padded): store after init
        for b in range(NB):
            if last_phase[b] < 0:
                rows = int(bsz[b])
                nc.sync.dma_start(out=out_ext[b * 128:b * 128 + rows, :],
                                  in_=out_acc[0:rows, b * 128:b * 128 + 128])

    nc.finalize()
    return nc


# ---------------------------------------------------------------- runners

def prep_all(inputs, cfg: Cfg):
    in_maps, meta = host_prep(inputs["x"], inputs["edge_index"],
                              inputs["edge_weight"], cfg)
    consts = host_consts(inputs["W0"], inputs["b0"], inputs["W1"], inputs["b1"],
                         inputs["gamma0"], inputs["beta0"], inputs["mean0"],
                         inputs["var0"], inputs["gamma1"], inputs["beta1"],
                         inputs["mean1"], inputs["var1"], inputs["act_params"])
    for m in in_maps:
        m.update(consts)
    return in_maps, meta


def unshard(results, cfg: Cfg, meta=None):
    NL = cfg.N // cfg.P
    out = np.zeros((cfg.N, cfg.D), np.float32)
    for c in range(cfg.P):
        r = results[c]["out"]
        if meta is not None and "perms" in meta:
            out[c * NL:(c + 1) * NL] = r[meta["perms"][c]]
        else:
            out[c * NL:(c + 1) * NL] = r
    return out


# ---------------------------------------------------------------- entrypoint

def _install_dge_patch():
    """walrus needs --dge-levels=vector_dynamic_offsets for indirect DMAs."""
    from concourse import bass_utils as _bu
    if getattr(_bu, "_gcn_dge_patched", False):
        return
    _orig = _bu.run_command

    def _patched(argv, **kwargs):
        if argv and "walrus_driver" in str(argv[0]) and not any(
                str(a).startswith("--dge-levels") for a in argv):
            argv = list(argv) + ["--dge-levels=vector_dynamic_offsets"]
        return _orig(argv, **kwargs)

    _bu.run_command = _patched
    _bu._gcn_dge_patched = True


_CFG = Cfg()


def kernel(**inputs):
    """Full-input entrypoint: shard, run on 8 NeuronCores, gather output."""
    import numpy as np
    _install_dge_patch()
    inputs = {k: np.asarray(v) for k, v in inputs.items()}
    in_maps, meta = prep_all(inputs, _CFG)
    nc = build(meta, _CFG)
    res = run_bass_kernel_spmd(nc, in_maps, core_ids=list(range(_CFG.P)))
    return unshard([{k: np.asarray(v) for k, v in r.items()} for r in res.results],
                   _CFG, meta)
